# revision 1
# baseline (speedup 1.0000x reference)
"""Trainium2 Bass kernel for combined cross-entropy + batch-hard triplet loss.

Problem (N=4096, C=751, D=2048, 1024 identities x 4 instances):
  loss = mean(-log_softmax(logits)[i, t_i]) +
         mean(relu(max_same(dist) - min_diff(dist) + 0.5))
  with dist = pairwise Euclidean distances of feat rows.

Sharding: row-parallel. Core c computes the [512, 4096] block of the Gram
matrix for its rows via fp32r matmuls (FP22 multiply precision, full PE rate),
with -sq_i/2 - sq_j/2 - 65536*same(i,j) folded into the same PSUM accumulation
through one extra K=36 matmul per block:
  rows 0..31 : 256 * onehot(group_of(i))   x   -256 * onehot(group_of(j))
  row  32,33 : 1                           x   -sq_hi[j]/2 , -sq_lo[j]/2
  row  34,35 : -sq_hi[i]/2 , -sq_lo[i]/2   x   1
(256 = 2^8 and the hi/lo split keep everything exactly representable in FP22.)

Then per row: q = -2*psum = d2 + 131072*same, so
  hardest-negative^2 = -2 * max_j(psum)          (same-entries pushed far down)
  hardest-positive^2 = -2 * min_j(psum) - 131072 (same-entries pushed far up...
                        i.e. min picks the masked same entries)
Both come from plain DVE reduces directly on PSUM. Rows are pre-sorted by
target on the host (the loss is invariant to row permutation), which makes the
same-identity mask a fixed block-diagonal pattern of 4-row groups.

Each core also handles the cross entropy for its 512 rows (ACT exp with fused
row-sum, one-hot gather via scalar_tensor_tensor). Output per core: [128, 8]
(4 cols of per-row xent terms, 4 cols of per-row triplet terms, one col per
128-row tile). Host sums and averages.
"""

import sys

if "/opt/trn_rl_repo" not in sys.path:
    sys.path.insert(0, "/opt/trn_rl_repo")

import numpy as np

N = 4096
D = 2048
C = 751
NCORES = 8
RPC = N // NCORES          # rows per core = 512
MT = RPC // 128            # 128-row tiles per core = 4
NB = N // 512              # 512-wide column blocks = 8
KT = D // 128              # K chunks = 16
KF = 36                    # fold matmul contraction size
BIG = 131072.0             # 2^17: same-pair offset in q = -2*psum
MASK_SCALE = 256.0         # sqrt(BIG/2) = 2^8, exact in FP22
ALPHA = 1.0
BETA = 1.0
MARGIN = 0.5

_compiled = {}


def _build_nc():
    import concourse.bass as bass  # noqa: F401
    import concourse.tile as tile
    from concourse import mybir, bacc
    from contextlib import ExitStack

    f32 = mybir.dt.float32
    f32r = mybir.dt.float32r
    Alu = mybir.AluOpType
    Act = mybir.ActivationFunctionType
    X = mybir.AxisListType.X

    nc = bacc.Bacc("TRN2", target_bir_lowering=False, debug=False)

    fT = nc.dram_tensor("fT", [D, N], f32r, kind="ExternalInput").ap()
    lhsT = nc.dram_tensor("lhsT", [D, RPC], f32r, kind="ExternalInput").ap()
    fold_rhs = nc.dram_tensor("fold_rhs", [MT * KF, N], f32r, kind="ExternalInput").ap()
    fold_lhsT = nc.dram_tensor("fold_lhsT", [KF, RPC], f32r, kind="ExternalInput").ap()
    logits_in = nc.dram_tensor("logits", [RPC, C], f32, kind="ExternalInput").ap()
    onehot_in = nc.dram_tensor("onehot", [RPC, C], f32, kind="ExternalInput").ap()
    consts_in = nc.dram_tensor("consts", [128, 2], f32, kind="ExternalInput").ap()
    out_dram = nc.dram_tensor("out", [128, 8], f32, kind="ExternalOutput").ap()

    with tile.TileContext(nc) as tc, ExitStack() as ctx:
        resident = ctx.enter_context(tc.tile_pool(name="resident", bufs=1))
        rhs_pool = ctx.enter_context(tc.tile_pool(name="rhs", bufs=32))
        fold_pool = ctx.enter_context(tc.tile_pool(name="fold", bufs=6))
        psum_pool = ctx.enter_context(tc.tile_pool(name="psum", bufs=8, space="PSUM"))
        xent_pool = ctx.enter_context(tc.tile_pool(name="xent", bufs=2))
        small_pool = ctx.enter_context(tc.tile_pool(name="small", bufs=2))

        # --- resident data ---
        lhsT_all = resident.tile([128, KT * RPC], f32r)   # k-chunk k at cols [RPC*k, RPC*(k+1))
        for k in range(KT):
            nc.sync.dma_start(lhsT_all[:, bass.ts(k, RPC)], lhsT[bass.ts(k, 128), :])
        flh = resident.tile([KF, RPC], f32r)
        nc.sync.dma_start(flh[:], fold_lhsT[:])
        consts = resident.tile([128, 2], f32)
        nc.sync.dma_start(consts[:], consts_in[:])
        neg_big = consts[:, 0:1]
        margin = consts[:, 1:2]

        mx_slots = [resident.tile([128, NB], f32, tag=f"mxs{m}", name=f"mxs{m}") for m in range(MT)]
        mn_slots = [resident.tile([128, NB], f32, tag=f"mns{m}", name=f"mns{m}") for m in range(MT)]
        out_tile = resident.tile([128, 8], f32)

        # --- main GEMM + mining ---
        for n in range(NB):
            rhs_tiles = []
            for k in range(KT):
                rt = rhs_pool.tile([128, 512], f32r, tag="rhs")
                nc.sync.dma_start(rt[:], fT[bass.ts(k, 128), bass.ts(n, 512)])
                rhs_tiles.append(rt)
            for m in range(MT):
                fr = fold_pool.tile([KF, 512], f32r, tag="fr")
                nc.sync.dma_start(fr[:], fold_rhs[bass.ds(m * KF, KF), bass.ts(n, 512)])
                ps = psum_pool.tile([128, 512], mybir.dt.float32, tag="ps")
                for k in range(KT):
                    nc.tensor.matmul(
                        ps[:],
                        lhsT_all[:, bass.ds(RPC * k + 128 * m, 128)],
                        rhs_tiles[k][:],
                        start=(k == 0),
                        stop=False,
                    )
                nc.tensor.matmul(ps[:], flh[:, bass.ts(m, 128)], fr[:],
                                 start=False, stop=True)
                nc.vector.tensor_reduce(mx_slots[m][:, n:n + 1], ps[:], axis=X, op=Alu.max)
                nc.vector.tensor_reduce(mn_slots[m][:, n:n + 1], ps[:], axis=X, op=Alu.min)

        # --- triplet tails ---
        for m in range(MT):
            t_an = small_pool.tile([128, 1], f32, tag="t_an")
            t_ap = small_pool.tile([128, 1], f32, tag="t_ap")
            nc.vector.tensor_reduce(t_an[:], mx_slots[m][:], axis=X, op=Alu.max)
            nc.vector.tensor_reduce(t_ap[:], mn_slots[m][:], axis=X, op=Alu.min)
            d_an = small_pool.tile([128, 1], f32, tag="d_an")
            d_ap = small_pool.tile([128, 1], f32, tag="d_ap")
            # dist_an = sqrt(-2 * t_an) ; dist_ap = sqrt(-2 * t_ap - BIG)
            nc.scalar.activation(d_an[:], t_an[:], Act.Sqrt, scale=-2.0)
            nc.scalar.activation(d_ap[:], t_ap[:], Act.Sqrt, bias=neg_big, scale=-2.0)
            diff = small_pool.tile([128, 1], f32, tag="diff")
            nc.vector.tensor_sub(diff[:], d_ap[:], d_an[:])
            nc.scalar.activation(out_tile[:, 4 + m:5 + m], diff[:], Act.Relu,
                                 bias=margin, scale=1.0)

        # --- cross entropy ---
        for r in range(MT):
            lg = xent_pool.tile([128, C], f32, tag="lg")
            oh = xent_pool.tile([128, C], f32, tag="oh")
            nc.sync.dma_start(lg[:], logits_in[bass.ts(r, 128), :])
            nc.sync.dma_start(oh[:], onehot_in[bass.ts(r, 128), :])
            mx = small_pool.tile([128, 1], f32, tag="xmx")
            nc.vector.tensor_reduce(mx[:], lg[:], axis=X, op=Alu.max)
            negmx = small_pool.tile([128, 1], f32, tag="negmx")
            nc.vector.tensor_scalar_mul(negmx[:], mx[:], -1.0)
            escr = xent_pool.tile([128, C], f32, tag="escr")
            s = small_pool.tile([128, 1], f32, tag="s")
            nc.scalar.activation(escr[:], lg[:], Act.Exp, bias=negmx[:], scale=1.0,
                                 accum_out=s[:])
            gscr = xent_pool.tile([128, C], f32, tag="gscr")
            tv = small_pool.tile([128, 1], f32, tag="tv")
            nc.vector.scalar_tensor_tensor(out=gscr[:], in0=lg[:], scalar=1.0,
                                           in1=oh[:], op0=Alu.mult, op1=Alu.mult,
                                           accum_out=tv[:])
            l1 = small_pool.tile([128, 1], f32, tag="l1")
            nc.scalar.activation(l1[:], s[:], Act.Ln, scale=1.0)
            # xent_row = (l1 + mx) - tv
            nc.vector.scalar_tensor_tensor(out=out_tile[:, r:r + 1], in0=l1[:],
                                           scalar=mx[:], in1=tv[:],
                                           op0=Alu.add, op1=Alu.subtract)

        nc.sync.dma_start(out_dram[:], out_tile[:])

    nc.compile()
    return nc


def _fp22_hi(v):
    return (np.ascontiguousarray(v, dtype=np.float32).view(np.uint32)
            & np.uint32(0xFFFFFC00)).view(np.float32)


def _prepare(logits, feat, targets):
    logits = np.asarray(logits, dtype=np.float32)
    feat = np.asarray(feat, dtype=np.float32)
    targets = np.asarray(targets)

    perm = np.argsort(targets, kind="stable")
    t = np.asarray(targets)[perm]
    tg = t.reshape(-1, 4)
    assert (tg == tg[:, :1]).all(), "expected PK sampling with 4 instances/identity"

    feat_p = feat[perm]
    logits_p = logits[perm]

    fT = np.ascontiguousarray(feat_p.T)                      # [D, N]
    sq = np.einsum("ij,ij->i", feat_p.astype(np.float64), feat_p.astype(np.float64))
    sq = sq.astype(np.float32)
    sq_hi = _fp22_hi(sq)
    sq_lo = (sq - sq_hi).astype(np.float32)

    # fold_lhsT [KF, RPC] per core: rows 0..31 structural mask (identical for
    # every core), rows 32,33 ones, rows 34,35 -sq_hi/2, -sq_lo/2 of own rows.
    mask_pat = np.zeros((32, RPC), dtype=np.float32)
    idx = np.arange(RPC)
    mask_pat[(idx % 128) // 4, idx] = MASK_SCALE

    in_maps = []
    for c in range(NCORES):
        rows = slice(c * RPC, (c + 1) * RPC)
        flh = np.zeros((KF, RPC), dtype=np.float32)
        flh[:32] = mask_pat
        flh[32] = 1.0
        flh[33] = 1.0
        flh[34] = -0.5 * sq_hi[rows]
        flh[35] = -0.5 * sq_lo[rows]

        frh = np.zeros((MT * KF, N), dtype=np.float32)
        for m in range(MT):
            blk = frh[m * KF:(m + 1) * KF]
            # group g of m-tile m covers columns c*RPC + 128*m + 4*g ... +4
            base = c * RPC + 128 * m
            for g in range(32):
                blk[g, base + 4 * g: base + 4 * g + 4] = -MASK_SCALE
            blk[32] = -0.5 * sq_hi
            blk[33] = -0.5 * sq_lo
            blk[34] = 1.0
            blk[35] = 1.0

        # match jax gather semantics: negative indices wrap, then clamp
        ti = t[rows].astype(np.int64)
        ti = np.where(ti < 0, ti + C, ti)
        ti = np.clip(ti, 0, C - 1)
        oh = np.zeros((RPC, C), dtype=np.float32)
        oh[np.arange(RPC), ti] = 1.0

        consts = np.zeros((128, 2), dtype=np.float32)
        consts[:, 0] = -BIG
        consts[:, 1] = MARGIN

        in_maps.append({
            "fT": fT,
            "lhsT": np.ascontiguousarray(fT[:, rows]),
            "fold_rhs": frh,
            "fold_lhsT": flh,
            "logits": np.ascontiguousarray(logits_p[rows]),
            "onehot": oh,
            "consts": consts,
        })
    return in_maps


def _combine(results):
    xent_sum = 0.0
    trip_sum = 0.0
    for r in results:
        o = r["out"].astype(np.float64)
        xent_sum += o[:, :4].sum()
        trip_sum += o[:, 4:].sum()
    loss = ALPHA * (xent_sum / N) + BETA * (trip_sum / N)
    return np.float32(loss)


def kernel(logits, feat, targets):
    from concourse.bass_utils import run_bass_kernel_spmd

    if "nc" not in _compiled:
        _compiled["nc"] = _build_nc()
    nc = _compiled["nc"]

    in_maps = _prepare(logits, feat, targets)
    res = run_bass_kernel_spmd(nc, in_maps, core_ids=list(range(NCORES)))
    return _combine(res.results)



# revision 6
# speedup vs baseline: 1.5404x; 1.5404x over previous
"""Trainium2 Bass kernel for combined cross-entropy + batch-hard triplet loss.

Problem (N=4096, C=751, D=2048, 1024 identities x 4 instances):
  loss = mean(-log_softmax(logits)[i, t_i]) +
         mean(relu(max_same(dist) - min_diff(dist) + 0.5))
  with dist = pairwise Euclidean distances of feat rows.

v2: symmetric tiling. The Gram/distance matrix is symmetric, so only the
upper-triangle tiles are computed: at [128-row x 512-col] granularity there
are 144 kept tiles (of 256), split 18 per core with a uniform slot pattern
(4+4+2+8 tiles across at most two distinct 512-col blocks per core; the 4
diagonal-band tiles always sit at t=0..3). All matmuls run in bf16 (full PE
rate, half the HBM traffic of fp32). Per tile the PSUM accumulates
  psum = feat_i . feat_j - sq_i/2 - sq_j/2 - 65536*same(i,j)
via 16 K=128 matmuls plus one K=36 "fold" matmul (mask rows 256 x -256,
sq hi/lo split rows so everything is bf16-exact). Mining:
  row-side   : DVE max (hardest negative) on every tile, DVE min (hardest
               positive) on the 4 diagonal tiles only.
  column-side: for off-diagonal tiles the transposed coverage is obtained by
               ACT psum->SBUF copy, PE transpose (f32r), DVE max over the
               transposed chunks -> per-column partials.
The host combines the small partial outputs: per-row max over row/col-side
partials -> dist_an, diagonal min -> dist_ap, then sqrt/relu/mean in f64.
Cross entropy: device computes row-wise log-sum-exp of the (bf16) logits
(ACT Exp with fused accumulation + Ln); host subtracts the gathered target
logit. Rows are pre-sorted by target on the host (loss is permutation
invariant) so same-identity groups are 4 consecutive rows.
"""

import sys

if "/opt/trn_rl_repo" not in sys.path:
    sys.path.insert(0, "/opt/trn_rl_repo")

import numpy as np
import ml_dtypes

BF16 = ml_dtypes.bfloat16

N = 4096
D = 2048
C = 751
NCORES = 8
RPC = N // NCORES          # xent rows per core = 512
NT = 18                    # distance tiles per core
KT = D // 128              # K chunks = 16
KF = 36                    # fold matmul contraction size
BIG = 131072.0             # 2^17: same-pair offset in q = -2*psum
MASK_SCALE = 256.0         # sqrt(BIG/2) = 2^8, exact in bf16
ALPHA = 1.0
BETA = 1.0
MARGIN = 0.5

# --- static tile assignment -------------------------------------------------
# Kept tiles: (row_tile r in 0..31, col_block c in 0..7) with r <= 4c+3.
# Slot sizes per core: [4, 4, 2, 8]; slot0 always holds the 4 diagonal-band
# tiles (r in 4c..4c+3). Each core touches at most 2 distinct col blocks.
SLOT_SIZES = [4, 4, 2, 8]
SLOT_OF_T = [0] * 4 + [1] * 4 + [2] * 2 + [3] * 8
ASSIGN = [
    [(0, [0, 1, 2, 3]), (7, [0, 1, 2, 3]), (7, [4, 5]), (7, list(range(6, 14)))],
    [(7, [28, 29, 30, 31]), (7, [14, 15, 16, 17]), (7, [18, 19]), (7, list(range(20, 28)))],
    [(1, [4, 5, 6, 7]), (1, [0, 1, 2, 3]), (6, [0, 1]), (6, list(range(2, 10)))],
    [(6, [24, 25, 26, 27]), (6, [10, 11, 12, 13]), (6, [14, 15]), (6, list(range(16, 24)))],
    [(2, [8, 9, 10, 11]), (5, [0, 1, 2, 3]), (5, [4, 5]), (2, list(range(0, 8)))],
    [(5, [20, 21, 22, 23]), (5, [6, 7, 8, 9]), (5, [10, 11]), (5, list(range(12, 20)))],
    [(3, [12, 13, 14, 15]), (3, [0, 1, 2, 3]), (4, [0, 1]), (3, list(range(4, 12)))],
    [(4, [16, 17, 18, 19]), (4, [2, 3, 4, 5]), (4, [6, 7]), (4, list(range(8, 16)))],
]

# TILES[c] = [(row_tile, col_block)] * 18, diag tiles at t=0..3
TILES = []
for _c in range(NCORES):
    _tl = []
    for _cb, _rows in ASSIGN[_c]:
        _tl.extend((_r, _cb) for _r in _rows)
    TILES.append(_tl)

# SLOT_BLOCK[c][s] = col block resident in slot s for core c
SLOT_BLOCK = [[cb for cb, _ in ASSIGN[c]] for c in range(NCORES)]

# TILE_AT[(r, cb)] = (core, t)
TILE_AT = {}
for _c in range(NCORES):
    for _t, (_r, _cb) in enumerate(TILES[_c]):
        assert (_r, _cb) not in TILE_AT
        TILE_AT[(_r, _cb)] = (_c, _t)

# sanity: full upper-triangle coverage, diag placement
assert len(TILE_AT) == 144
for _r in range(32):
    for _cb in range(_r // 4, 8):
        assert (_r, _cb) in TILE_AT
for _c in range(NCORES):
    for _t, (_r, _cb) in enumerate(TILES[_c]):
        assert (_t < 4) == (_r // 4 == _cb), (_c, _t, _r, _cb)

# out1 column layout
O_RMAX = 0                 # cols 0..17  : row-side max per tile
O_RMIN = NT                # cols 18..21 : row-side min, diag tiles t=0..3
O_LSE = NT + 4             # cols 22..25 : xent log-sum-exp per 128-row tile
O_W = NT + 8               # 26 cols

_compiled = {}


def _build_nc():
    import concourse.bass as bass  # noqa: F401
    import concourse.tile as tile
    from concourse import mybir, bacc
    from contextlib import ExitStack

    f32 = mybir.dt.float32
    f32r = mybir.dt.float32r
    bf16 = mybir.dt.bfloat16
    Alu = mybir.AluOpType
    Act = mybir.ActivationFunctionType
    X = mybir.AxisListType.X

    nc = bacc.Bacc("TRN2", target_bir_lowering=False, debug=False)

    rhs_in = nc.dram_tensor("rhs_pack", [KT, 128, 2048], bf16, kind="ExternalInput").ap()
    lhs_in = nc.dram_tensor("lhs_pack", [NT, 128, 2048], bf16, kind="ExternalInput").ap()
    flh_in = nc.dram_tensor("fold_lhs", [KF, NT * 128], bf16, kind="ExternalInput").ap()
    frh_in = nc.dram_tensor("fold_rhs", [NT, KF, 512], bf16, kind="ExternalInput").ap()
    logits_in = nc.dram_tensor("logits", [RPC, C], bf16, kind="ExternalInput").ap()
    ident_in = nc.dram_tensor("ident", [128, 128], f32r, kind="ExternalInput").ap()
    out1_dram = nc.dram_tensor("out1", [128, O_W], f32, kind="ExternalOutput").ap()
    out2_dram = nc.dram_tensor("out2", [NT, 128, 4], f32, kind="ExternalOutput").ap()

    with tile.TileContext(nc) as tc, ExitStack() as ctx:
        resident = ctx.enter_context(tc.tile_pool(name="resident", bufs=1))
        lhs_pool = ctx.enter_context(tc.tile_pool(name="lhs", bufs=4))
        fr_pool = ctx.enter_context(tc.tile_pool(name="fr", bufs=4))
        sb_pool = ctx.enter_context(tc.tile_pool(name="sbt", bufs=3))
        ps_pool = ctx.enter_context(tc.tile_pool(name="ps", bufs=4, space="PSUM"))
        pt_pool = ctx.enter_context(tc.tile_pool(name="pt", bufs=3, space="PSUM"))
        xent_pool = ctx.enter_context(tc.tile_pool(name="xent", bufs=2))
        small_pool = ctx.enter_context(tc.tile_pool(name="small", bufs=4))

        # --- resident data ---
        ident = resident.tile([128, 128], f32r)
        nc.sync.dma_start(ident[:], ident_in[:])

        rhs_k = []
        for k in range(KT):
            rk = resident.tile([128, 2048], bf16, tag=f"rhs{k}", name=f"rhs{k}")
            nc.sync.dma_start(rk[:], rhs_in[k])
            rhs_k.append(rk)
        flh = resident.tile([KF, NT * 128], bf16)
        nc.sync.dma_start(flh[:], flh_in[:])

        out_tile = resident.tile([128, O_W], f32)

        # --- cross entropy: lse per row (overlaps with resident rhs load) ---
        for x in range(4):
            lg = xent_pool.tile([128, C], bf16, tag="lg")
            nc.sync.dma_start(lg[:], logits_in[bass.ts(x, 128), :])
            escr = xent_pool.tile([128, C], bf16, tag="escr")
            s = small_pool.tile([128, 1], f32, tag="s")
            nc.scalar.activation(escr[:], lg[:], Act.Exp, accum_out=s[:])
            nc.scalar.activation(out_tile[:, O_LSE + x:O_LSE + x + 1], s[:],
                                 Act.Ln, scale=1.0)

        # --- main loop: 18 tiles, software-pipelined transposes ---
        sb_tiles = [None] * NT

        def emit_transpose(t):
            sb = sb_tiles[t]
            pt = pt_pool.tile([128, 4, 128], f32r, tag="pt")
            for i in range(4):
                nc.tensor.transpose(pt[:, i, :], sb[:, bass.ts(i, 128)], ident[:])
            cm = small_pool.tile([128, 4], f32, tag="cm")
            nc.vector.tensor_reduce(cm[:], pt[:], axis=X, op=Alu.max)
            nc.sync.dma_start(out2_dram[t], cm[:])

        for t in range(NT):
            s = SLOT_OF_T[t]
            lhs_t = lhs_pool.tile([128, 2048], bf16, tag="lhs")
            nc.sync.dma_start(lhs_t[:], lhs_in[t])
            fr = fr_pool.tile([KF, 512], bf16, tag="fr")
            nc.sync.dma_start(fr[:], frh_in[t])

            ps = ps_pool.tile([128, 512], f32, tag="ps")
            for k in range(KT):
                nc.tensor.matmul(
                    ps[:],
                    lhs_t[:, bass.ts(k, 128)],
                    rhs_k[k][:, bass.ts(s, 512)],
                    start=(k == 0),
                    stop=False,
                )
            nc.tensor.matmul(ps[:], flh[:, bass.ts(t, 128)], fr[:],
                             start=False, stop=True)

            nc.vector.tensor_reduce(out_tile[:, O_RMAX + t:O_RMAX + t + 1],
                                    ps[:], axis=X, op=Alu.max)
            if t < 4:
                nc.vector.tensor_reduce(out_tile[:, O_RMIN + t:O_RMIN + t + 1],
                                        ps[:], axis=X, op=Alu.min)
            else:
                sb = sb_pool.tile([128, 512], f32r, tag="sb")
                nc.scalar.activation(sb[:], ps[:], Act.Copy, scale=1.0)
                sb_tiles[t] = sb
                # transpose of tile t-1 lands behind tile t's matmuls so the
                # PE never waits on the ACT copy
                if t >= 5:
                    emit_transpose(t - 1)
        emit_transpose(NT - 1)

        nc.sync.dma_start(out1_dram[:], out_tile[:])

    nc.compile()
    return nc


def _bf16_hi_lo(v):
    """Split f32 vector into two bf16-exact f32 parts (hi + lo ~= v)."""
    hi = np.asarray(v, dtype=np.float32).astype(BF16).astype(np.float32)
    lo = (np.asarray(v, dtype=np.float32) - hi).astype(BF16).astype(np.float32)
    return hi, lo


def _prepare(logits, feat, targets):
    logits = np.asarray(logits, dtype=np.float32)
    feat = np.asarray(feat, dtype=np.float32)
    targets = np.asarray(targets)

    perm = np.argsort(targets, kind="stable")
    t_sorted = np.asarray(targets)[perm]
    tg = t_sorted.reshape(-1, 4)
    assert (tg == tg[:, :1]).all(), "expected PK sampling with 4 instances/identity"

    F = feat[perm].astype(BF16)                  # [N, D] bf16
    FT = np.ascontiguousarray(F.T)               # [D, N] bf16
    F64 = F.astype(np.float64)
    sq = np.einsum("ij,ij->i", F64, F64).astype(np.float32)
    sq_hi, sq_lo = _bf16_hi_lo(sq)

    logits_p = logits[perm].astype(BF16)

    # mask patterns (bf16-exact values, stored f32 then cast)
    mask_lhs = np.zeros((32, 128), dtype=np.float32)
    m_idx = np.arange(128)
    mask_lhs[m_idx // 4, m_idx] = MASK_SCALE

    FT3 = FT.reshape(KT, 128, N)

    in_maps = []
    for c in range(NCORES):
        tiles = TILES[c]

        # rhs_pack [KT, 128, 2048]: slot s cols = block SLOT_BLOCK[c][s]
        cols = np.concatenate([np.arange(512 * SLOT_BLOCK[c][s],
                                         512 * SLOT_BLOCK[c][s] + 512)
                               for s in range(4)])
        rhs_pack = np.ascontiguousarray(FT3[:, :, cols])

        # lhs_pack [NT, 128, 2048]: tile t -> [p, 128k+m] = FT[128k+p, rows_t[m]]
        lhs_pack = np.empty((NT, 128, 2048), dtype=BF16)
        for t, (r, _cb) in enumerate(tiles):
            blk = FT3[:, :, 128 * r:128 * r + 128]      # [KT, 128p, 128m]
            lhs_pack[t] = blk.transpose(1, 0, 2).reshape(128, 2048)

        # fold_lhs [KF, NT*128]
        flh = np.zeros((KF, NT * 128), dtype=np.float32)
        for t, (r, _cb) in enumerate(tiles):
            cs = slice(128 * t, 128 * t + 128)
            flh[:32, cs] = mask_lhs
            flh[32, cs] = 1.0
            flh[33, cs] = 1.0
            rows = slice(128 * r, 128 * r + 128)
            flh[34, cs] = -0.5 * sq_hi[rows]
            flh[35, cs] = -0.5 * sq_lo[rows]

        # fold_rhs [NT, KF, 512]
        frh = np.zeros((NT, KF, 512), dtype=np.float32)
        for t, (r, cb) in enumerate(tiles):
            cols_blk = slice(512 * cb, 512 * cb + 512)
            frh[t, 32] = -0.5 * sq_hi[cols_blk]
            frh[t, 33] = -0.5 * sq_lo[cols_blk]
            frh[t, 34] = 1.0
            frh[t, 35] = 1.0
            if t < 4:  # diagonal-band tile: same-pair mask
                base = 128 * (r % 4)
                for g in range(32):
                    frh[t, g, base + 4 * g: base + 4 * g + 4] = -MASK_SCALE

        in_maps.append({
            "rhs_pack": rhs_pack,
            "lhs_pack": lhs_pack,
            "fold_lhs": flh.astype(BF16),
            "fold_rhs": frh.astype(BF16),
            "logits": np.ascontiguousarray(logits_p[c * RPC:(c + 1) * RPC]),
            "ident": np.eye(128, dtype=np.float32),
        })

    # stash for _combine
    _prepare.cache = {
        "logits_p_bf": logits_p.astype(np.float64),
        "t_sorted": t_sorted,
    }
    return in_maps


def _combine(results):
    cache = _prepare.cache
    out1 = [np.asarray(r["out1"], dtype=np.float64) for r in results]
    out2 = [np.asarray(r["out2"], dtype=np.float64) for r in results]

    # --- triplet ---
    qmax = np.empty(N)
    qmin = np.empty(N)
    for rt in range(32):
        R = rt // 4
        rows = slice(128 * rt, 128 * rt + 128)
        parts = []
        for cb in range(R, 8):                       # row-side partials
            c, t = TILE_AT[(rt, cb)]
            parts.append(out1[c][:, O_RMAX + t])
        for rp in range(0, 4 * R):                   # col-side partials
            c, t = TILE_AT[(rp, R)]
            parts.append(out2[c][t][:, rt - 4 * R])
        qmax[rows] = np.max(np.stack(parts), axis=0)
        c, t = TILE_AT[(rt, R)]
        qmin[rows] = out1[c][:, O_RMIN + t]

    d2_an = np.maximum(-2.0 * qmax, 1e-12)
    d2_ap = np.maximum(-2.0 * qmin - BIG, 1e-12)
    dist_an = np.sqrt(d2_an)
    dist_ap = np.sqrt(d2_ap)
    trip = np.mean(np.maximum(dist_ap - dist_an + MARGIN, 0.0))

    # --- cross entropy ---
    lse = np.empty(N)
    for c in range(NCORES):
        for x in range(4):
            lse[c * RPC + 128 * x: c * RPC + 128 * (x + 1)] = \
                out1[c][:, O_LSE + x]
    ti = cache["t_sorted"].astype(np.int64)
    ti = np.where(ti < 0, ti + C, ti)
    ti = np.clip(ti, 0, C - 1)
    gathered = cache["logits_p_bf"][np.arange(N), ti]
    xent = np.mean(lse - gathered)

    return np.float32(ALPHA * xent + BETA * trip)


def kernel(logits, feat, targets):
    from concourse.bass_utils import run_bass_kernel_spmd

    if "nc" not in _compiled:
        _compiled["nc"] = _build_nc()
    nc = _compiled["nc"]

    in_maps = _prepare(logits, feat, targets)
    res = run_bass_kernel_spmd(nc, in_maps, core_ids=list(range(NCORES)))
    return _combine(res.results)
